# revision 4
# baseline (speedup 1.0000x reference)
"""Trainium2 Bass kernel for BaseSOM forward (vq_codebook).

For batch [4096, 512] and codebook weights [4096, 512] (64x64 SOM grid):
  1. bmu(i) = argmin_j ||batch_i - w_j||^2   (== argmax_j b.w_j - 0.5||w_j||^2)
  2. out[i, j] = exp(-grid_dist(j, bmu(i))^2 / sigma_op^2)

Sharding: data-parallel over batch across 8 NeuronCores (512 rows/core),
weights replicated.  Host marshaling pre-transposes batch/weights so the
contraction dim lands on SBUF partitions.

Precision: TRN2's full-rate fp32 matmul dtype (float32r) rounds inputs to 12
mantissa bits, which flips ~10 argmins on this data.  We therefore run a
compensated 3-term product: with bh = R(b), bl = R(b - bh) (R = fp32r
rounding, done host-side so on-chip values are exact):
    b.w  ~=  bh.wh + bh.wl + bl.wh      (error ~2^-26, 0 flips, min gap 1e-4)
-0.5||w||^2 is folded in as a K=2 rank-2 update (hi+lo rows against ones).

Per 128-row batch tile and 2048-wide j-half: 13 fp32r matmuls accumulate
scores into PSUM; DVE max/max_index give the argmax per half, merged into a
running (max, idx); then idx>>6 / idx&63 give the BMU grid coords, ScalarE
Square+Exp build 64-wide row/col Gaussians, and one broadcast tensor_tensor
multiply expands their outer product into the [128, 4096] output tile.
"""

import math

import numpy as np

import concourse.bass as bass
import concourse.tile as tile
from concourse import bacc, mybir
from concourse.bass_utils import run_bass_kernel_spmd

N_CORES = 8
B = 4096
DIM = 512
MN = 4096
GRID = 64
B_SHARD = B // N_CORES
SIGMA = GRID / 2.0
HALF = MN // 2

F32 = mybir.dt.float32
F32R = mybir.dt.float32r
U32 = mybir.dt.uint32

_NC_CACHE = {}


def fp32r_round(a):
    """Round f32 array to fp32r (12 explicit mantissa bits, RNE) — bit-exact
    with walrus fp32_to_fp32r."""
    a = np.ascontiguousarray(a, dtype=np.float32)
    bits = a.view(np.uint32)
    low = bits & np.uint32(0xFFF)
    lsb = (bits >> np.uint32(12)) & np.uint32(1)
    add = ((low > 0x800) | ((low == 0x800) & (lsb == 1))).astype(np.uint32)
    out = (((bits >> np.uint32(12)) + add) << np.uint32(12)).astype(np.uint32)
    return out.view(np.float32).reshape(a.shape)


def _build_kernel(inv_sig2: float):
    nc = bacc.Bacc("TRN2", target_bir_lowering=False, debug=False)

    bh_d = nc.dram_tensor("bh", [DIM, B_SHARD], F32R, kind="ExternalInput").ap()
    bl_d = nc.dram_tensor("bl", [DIM, B_SHARD], F32R, kind="ExternalInput").ap()
    wh_d = nc.dram_tensor("wh", [DIM, MN], F32R, kind="ExternalInput").ap()
    wl_d = nc.dram_tensor("wl", [DIM, MN], F32R, kind="ExternalInput").ap()
    w2_d = nc.dram_tensor("w2", [2, MN], F32R, kind="ExternalInput").ap()
    aa_d = nc.dram_tensor("aa", [128, GRID], F32, kind="ExternalInput").ap()
    ones_d = nc.dram_tensor("ones", [2, 128], F32R, kind="ExternalInput").ap()
    out_d = nc.dram_tensor("out", [B_SHARD, MN], F32, kind="ExternalOutput").ap()

    n_tiles = B_SHARD // 128  # 4
    n_k = DIM // 128  # 4

    with tile.TileContext(nc) as tc:
        with (
            tc.tile_pool(name="consts", bufs=1) as consts,
            tc.tile_pool(name="wstream", bufs=2) as wstream,
            tc.tile_pool(name="psum", bufs=2, space="PSUM") as psum,
            tc.tile_pool(name="scr", bufs=2) as scr,
            tc.tile_pool(name="best", bufs=1) as best,
            tc.tile_pool(name="outp", bufs=2) as outp,
        ):
            bh = []
            bl = []
            for k in range(n_k):
                t = consts.tile([128, B_SHARD], F32R, tag=f"bh{k}")
                nc.sync.dma_start(t[:], bh_d[k * 128 : (k + 1) * 128, :])
                bh.append(t)
                t = consts.tile([128, B_SHARD], F32R, tag=f"bl{k}")
                nc.sync.dma_start(t[:], bl_d[k * 128 : (k + 1) * 128, :])
                bl.append(t)
            w2 = consts.tile([2, MN], F32R, tag="w2")
            nc.sync.dma_start(w2[:], w2_d[:, :])
            aa = consts.tile([128, GRID], F32, tag="aa")
            nc.sync.dma_start(aa[:], aa_d[:, :])
            ones = consts.tile([2, 128], F32R, tag="ones")
            nc.sync.dma_start(ones[:], ones_d[:, :])

            rmax = []
            ridx = []
            for m in range(n_tiles):
                t_rmax = best.tile([128, 1], F32, tag=f"rmax{m}")
                rmax.append(t_rmax)
                t_ridx = best.tile([128, 1], F32, tag=f"ridx{m}")
                ridx.append(t_ridx)

            for h in range(2):
                hsl = slice(h * HALF, (h + 1) * HALF)
                wh = []
                wl = []
                for k in range(n_k):
                    t = wstream.tile([128, HALF], F32R, tag=f"wh{k}")
                    nc.sync.dma_start(t[:], wh_d[k * 128 : (k + 1) * 128, hsl])
                    wh.append(t)
                    t = wstream.tile([128, HALF], F32R, tag=f"wl{k}")
                    nc.sync.dma_start(t[:], wl_d[k * 128 : (k + 1) * 128, hsl])
                    wl.append(t)

                for m in range(n_tiles):
                    msl = slice(m * 128, (m + 1) * 128)
                    sc = psum.tile([128, HALF], F32, tag="sc")
                    for nb in range(HALF // 512):
                        osl = slice(nb * 512, (nb + 1) * 512)
                        for k in range(n_k):
                            nc.tensor.matmul(
                                sc[:, osl], bh[k][:, msl], wh[k][:, osl],
                                start=(k == 0), stop=False, skip_group_check=True,
                            )
                        for k in range(n_k):
                            nc.tensor.matmul(
                                sc[:, osl], bh[k][:, msl], wl[k][:, osl],
                                start=False, stop=False, skip_group_check=True,
                            )
                        for k in range(n_k):
                            nc.tensor.matmul(
                                sc[:, osl], bl[k][:, msl], wh[k][:, osl],
                                start=False, stop=False, skip_group_check=True,
                            )
                        nc.tensor.matmul(
                            sc[:, osl], ones[:, :], w2[:, h * HALF + nb * 512 : h * HALF + (nb + 1) * 512],
                            start=False, stop=True, skip_group_check=True,
                        )

                    mx = scr.tile([128, 8], F32, tag="mx")
                    nc.vector.max(mx[:], sc[:, :])
                    ix = scr.tile([128, 8], U32, tag="ix")
                    nc.vector.max_index(ix[:], mx[:], sc[:, :])

                    # merge into running best (idx as f32; exact for < 2^24)
                    if h == 0:
                        nc.vector.tensor_copy(rmax[m][:], mx[:, 0:1])
                        nc.vector.tensor_copy(ridx[m][:], ix[:, 0:1])
                    else:
                        ibf = scr.tile([128, 1], F32, tag="ibf")
                        nc.vector.tensor_scalar(
                            ibf[:], ix[:, 0:1], float(HALF), None, mybir.AluOpType.add
                        )
                        gt = scr.tile([128, 1], F32, tag="gt")
                        nc.vector.tensor_tensor(
                            gt[:], mx[:, 0:1], rmax[m][:], mybir.AluOpType.is_gt
                        )
                        dif = scr.tile([128, 1], F32, tag="dif")
                        nc.vector.tensor_tensor(
                            dif[:], ibf[:], ridx[m][:], mybir.AluOpType.subtract
                        )
                        sel = scr.tile([128, 1], F32, tag="sel")
                        nc.vector.tensor_tensor(
                            sel[:], dif[:], gt[:], mybir.AluOpType.mult
                        )
                        nc.vector.tensor_tensor(
                            ridx[m][:], sel[:], ridx[m][:], mybir.AluOpType.add
                        )

                        # expand phase for tile m (runs after final merge)
                        idxu = scr.tile([128, 1], U32, tag="idxu")
                        nc.vector.tensor_copy(idxu[:], ridx[m][:])
                        ru = scr.tile([128, 1], U32, tag="ru")
                        nc.vector.tensor_scalar(
                            ru[:], idxu[:], 6, None, mybir.AluOpType.logical_shift_right
                        )
                        cu = scr.tile([128, 1], U32, tag="cu")
                        nc.vector.tensor_scalar(
                            cu[:], idxu[:], 63, None, mybir.AluOpType.bitwise_and
                        )
                        nr = scr.tile([128, 1], F32, tag="nr")
                        nc.vector.tensor_scalar(
                            nr[:], ru[:], -1.0, None, mybir.AluOpType.mult
                        )
                        ncl = scr.tile([128, 1], F32, tag="ncl")
                        nc.vector.tensor_scalar(
                            ncl[:], cu[:], -1.0, None, mybir.AluOpType.mult
                        )

                        er = scr.tile([128, GRID], F32, tag="er")
                        nc.scalar.activation(
                            er[:], aa[:], mybir.ActivationFunctionType.Square,
                            bias=nr[:], scale=1.0,
                        )
                        nc.scalar.activation(
                            er[:], er[:], mybir.ActivationFunctionType.Exp,
                            scale=-inv_sig2,
                        )
                        ec = scr.tile([128, GRID], F32, tag="ec")
                        nc.scalar.activation(
                            ec[:], aa[:], mybir.ActivationFunctionType.Square,
                            bias=ncl[:], scale=1.0,
                        )
                        nc.scalar.activation(
                            ec[:], ec[:], mybir.ActivationFunctionType.Exp,
                            scale=-inv_sig2,
                        )

                        ot = outp.tile([128, MN], F32, tag="ot")
                        o3 = ot[:].rearrange("p (a b) -> p a b", a=GRID)
                        er_b = er[:].unsqueeze(2).broadcast_to([128, GRID, GRID])
                        ec_b = ec[:].unsqueeze(1).broadcast_to([128, GRID, GRID])
                        nc.vector.tensor_tensor(o3, er_b, ec_b, mybir.AluOpType.mult)

                        nc.sync.dma_start(out_d[msl, :], ot[:])

    nc.compile()
    return nc


def get_nc(inv_sig2: float):
    key = float(inv_sig2)
    if key not in _NC_CACHE:
        _NC_CACHE[key] = _build_kernel(key)
    return _NC_CACHE[key]


def prepare(batch, weights, locations, decay_rate, it):
    batch = np.asarray(batch, dtype=np.float32)
    weights = np.asarray(weights, dtype=np.float32)

    lr = math.exp(-float(it) / float(decay_rate))
    sigma_op = np.float32(SIGMA) * np.float32(lr)
    inv_sig2 = 1.0 / (float(sigma_op) * float(sigma_op))

    wT = weights.T  # [DIM, MN]
    wh = fp32r_round(wT)
    wl = fp32r_round(wT - wh)
    w2f = (-0.5 * (weights.astype(np.float64) ** 2).sum(axis=1)).astype(np.float32)
    w2h = fp32r_round(w2f)
    w2l = fp32r_round(w2f - w2h)
    w2 = np.stack([w2h, w2l], axis=0)  # [2, MN]
    aa = np.broadcast_to(np.arange(GRID, dtype=np.float32), (128, GRID)).copy()

    in_maps = []
    for c in range(N_CORES):
        sT = batch[c * B_SHARD : (c + 1) * B_SHARD, :].T  # [DIM, B_SHARD]
        sh = fp32r_round(sT)
        sl = fp32r_round(sT - sh)
        in_maps.append(
            {
                "bh": sh, "bl": sl, "wh": wh, "wl": wl, "w2": w2, "aa": aa,
                "ones": np.ones((2, 128), dtype=np.float32),
            }
        )
    return inv_sig2, in_maps


def run(inputs, **spmd_kwargs):
    inv_sig2, in_maps = prepare(**inputs)
    nc = get_nc(inv_sig2)
    res = run_bass_kernel_spmd(
        nc, in_maps, core_ids=list(range(N_CORES)), **spmd_kwargs
    )
    out = np.concatenate([r["out"] for r in res.results], axis=0)
    return out, res


def kernel(batch, weights, locations, decay_rate, it):
    out, _ = run(
        dict(
            batch=batch,
            weights=weights,
            locations=locations,
            decay_rate=decay_rate,
            it=it,
        )
    )
    return out


# revision 5
# speedup vs baseline: 1.0236x; 1.0236x over previous
"""Trainium2 Bass kernel for BaseSOM forward (vq_codebook).

For batch [4096, 512] and codebook weights [4096, 512] (64x64 SOM grid):
  1. bmu(i) = argmin_j ||batch_i - w_j||^2   (== argmax_j b.w_j - 0.5||w_j||^2)
  2. out[i, j] = exp(-grid_dist(j, bmu(i))^2 / sigma_op^2)

Sharding: data-parallel over batch across 8 NeuronCores (512 rows/core),
weights replicated.  Host marshaling pre-transposes batch/weights so the
contraction dim lands on SBUF partitions.

Precision: TRN2's full-rate fp32 matmul dtype (float32r) rounds inputs to 12
mantissa bits, which flips ~10 argmins on this data (min top-2 gap 1.3e-4).
We run a compensated product (R = fp32r rounding, host-side):
    bh = R(b), wh = R(w)                       main term, fp32r matmul
    bl16 = bf16(b - bh), wl16 = bf16(w - wh)   residuals
    b.w ~= bh.wh + bf16(bh).wl16 + bl16.bf16(wh)     (max err 2.4e-5, 0 flips)
-0.5||w||^2 enters as a K=3 rank-update of bf16 hi/mid/lo rows against ones.
fp32r matmuls stream at ~2 PE cycles/col (4-byte SBUF path) vs 1 for bf16,
so the corrections run in bf16 at half the cost of fp32r.

Per 128-row batch tile and 2048-wide j-half: 13 matmuls per 512-col block
accumulate scores into PSUM; DVE max/max_index give the argmax per half,
merged into a running (max, idx); then idx>>6 / idx&63 give the BMU grid
coords, ScalarE Square+Exp build 64-wide row/col Gaussians, and one
broadcast tensor_tensor multiply expands their outer product into the
[128, 4096] output tile.
"""

import math

import ml_dtypes
import numpy as np

import concourse.bass as bass
import concourse.tile as tile
from concourse import bacc, mybir
from concourse.bass_utils import run_bass_kernel_spmd

N_CORES = 8
B = 4096
DIM = 512
MN = 4096
GRID = 64
B_SHARD = B // N_CORES
SIGMA = GRID / 2.0
HALF = MN // 2

F32 = mybir.dt.float32
F32R = mybir.dt.float32r
BF16 = mybir.dt.bfloat16
U32 = mybir.dt.uint32

_NC_CACHE = {}


def fp32r_round(a):
    """Round f32 array to fp32r (12 explicit mantissa bits, RNE) — bit-exact
    with walrus fp32_to_fp32r."""
    a = np.ascontiguousarray(a, dtype=np.float32)
    bits = a.view(np.uint32)
    low = bits & np.uint32(0xFFF)
    lsb = (bits >> np.uint32(12)) & np.uint32(1)
    add = ((low > 0x800) | ((low == 0x800) & (lsb == 1))).astype(np.uint32)
    out = (((bits >> np.uint32(12)) + add) << np.uint32(12)).astype(np.uint32)
    return out.view(np.float32).reshape(a.shape)


def bf16(a):
    return np.asarray(a, dtype=np.float32).astype(ml_dtypes.bfloat16)


def _build_kernel(inv_sig2: float):
    nc = bacc.Bacc("TRN2", target_bir_lowering=False, debug=False)

    bh_d = nc.dram_tensor("bh", [DIM, B_SHARD], F32R, kind="ExternalInput").ap()
    bh16_d = nc.dram_tensor("bh16", [DIM, B_SHARD], BF16, kind="ExternalInput").ap()
    bl16_d = nc.dram_tensor("bl16", [DIM, B_SHARD], BF16, kind="ExternalInput").ap()
    wh_d = nc.dram_tensor("wh", [DIM, MN], F32R, kind="ExternalInput").ap()
    wh16_d = nc.dram_tensor("wh16", [DIM, MN], BF16, kind="ExternalInput").ap()
    wl16_d = nc.dram_tensor("wl16", [DIM, MN], BF16, kind="ExternalInput").ap()
    w2_d = nc.dram_tensor("w2", [3, MN], BF16, kind="ExternalInput").ap()
    aa_d = nc.dram_tensor("aa", [128, GRID], F32, kind="ExternalInput").ap()
    ones_d = nc.dram_tensor("ones", [3, 128], BF16, kind="ExternalInput").ap()
    out_d = nc.dram_tensor("out", [B_SHARD, MN], F32, kind="ExternalOutput").ap()

    n_tiles = B_SHARD // 128  # 4
    n_k = DIM // 128  # 4

    with tile.TileContext(nc) as tc:
        with (
            tc.tile_pool(name="consts", bufs=1) as consts,
            tc.tile_pool(name="wstream", bufs=2) as wstream,
            tc.tile_pool(name="psum", bufs=2, space="PSUM") as psum,
            tc.tile_pool(name="scr", bufs=2) as scr,
            tc.tile_pool(name="best", bufs=1) as best,
            tc.tile_pool(name="outp", bufs=2) as outp,
        ):
            bh = []
            bh16 = []
            bl16 = []
            for k in range(n_k):
                ksl = slice(k * 128, (k + 1) * 128)
                t = consts.tile([128, B_SHARD], F32R, tag=f"bh{k}")
                nc.sync.dma_start(t[:], bh_d[ksl, :])
                bh.append(t)
                t = consts.tile([128, B_SHARD], BF16, tag=f"bh16{k}")
                nc.sync.dma_start(t[:], bh16_d[ksl, :])
                bh16.append(t)
                t = consts.tile([128, B_SHARD], BF16, tag=f"bl16{k}")
                nc.sync.dma_start(t[:], bl16_d[ksl, :])
                bl16.append(t)
            w2 = consts.tile([3, MN], BF16, tag="w2")
            nc.sync.dma_start(w2[:], w2_d[:, :])
            aa = consts.tile([128, GRID], F32, tag="aa")
            nc.sync.dma_start(aa[:], aa_d[:, :])
            ones = consts.tile([3, 128], BF16, tag="ones")
            nc.sync.dma_start(ones[:], ones_d[:, :])

            rmax = []
            ridx = []
            for m in range(n_tiles):
                t_rmax = best.tile([128, 1], F32, tag=f"rmax{m}")
                rmax.append(t_rmax)
                t_ridx = best.tile([128, 1], F32, tag=f"ridx{m}")
                ridx.append(t_ridx)

            for h in range(2):
                hsl = slice(h * HALF, (h + 1) * HALF)
                wh = []
                wh16 = []
                wl16 = []
                for k in range(n_k):
                    ksl = slice(k * 128, (k + 1) * 128)
                    t = wstream.tile([128, HALF], F32R, tag=f"wh{k}")
                    nc.sync.dma_start(t[:], wh_d[ksl, hsl])
                    wh.append(t)
                    t = wstream.tile([128, HALF], BF16, tag=f"wh16{k}")
                    nc.sync.dma_start(t[:], wh16_d[ksl, hsl])
                    wh16.append(t)
                    t = wstream.tile([128, HALF], BF16, tag=f"wl16{k}")
                    nc.sync.dma_start(t[:], wl16_d[ksl, hsl])
                    wl16.append(t)

                for m in range(n_tiles):
                    msl = slice(m * 128, (m + 1) * 128)
                    sc = psum.tile([128, HALF], F32, tag="sc")
                    for nb in range(HALF // 512):
                        osl = slice(nb * 512, (nb + 1) * 512)
                        for k in range(n_k):
                            nc.tensor.matmul(
                                sc[:, osl], bh[k][:, msl], wh[k][:, osl],
                                start=(k == 0), stop=False, skip_group_check=True,
                            )
                        for k in range(n_k):
                            nc.tensor.matmul(
                                sc[:, osl], bh16[k][:, msl], wl16[k][:, osl],
                                start=False, stop=False, skip_group_check=True,
                            )
                        for k in range(n_k):
                            nc.tensor.matmul(
                                sc[:, osl], bl16[k][:, msl], wh16[k][:, osl],
                                start=False, stop=False, skip_group_check=True,
                            )
                        nc.tensor.matmul(
                            sc[:, osl], ones[:, :],
                            w2[:, h * HALF + nb * 512 : h * HALF + (nb + 1) * 512],
                            start=False, stop=True, skip_group_check=True,
                        )

                    mx = scr.tile([128, 8], F32, tag="mx")
                    nc.vector.max(mx[:], sc[:, :])
                    ix = scr.tile([128, 8], U32, tag="ix")
                    nc.vector.max_index(ix[:], mx[:], sc[:, :])

                    # merge into running best (idx as f32; exact for < 2^24)
                    if h == 0:
                        nc.vector.tensor_copy(rmax[m][:], mx[:, 0:1])
                        nc.vector.tensor_copy(ridx[m][:], ix[:, 0:1])
                    else:
                        ibf = scr.tile([128, 1], F32, tag="ibf")
                        nc.vector.tensor_scalar(
                            ibf[:], ix[:, 0:1], float(HALF), None, mybir.AluOpType.add
                        )
                        gt = scr.tile([128, 1], F32, tag="gt")
                        nc.vector.tensor_tensor(
                            gt[:], mx[:, 0:1], rmax[m][:], mybir.AluOpType.is_gt
                        )
                        dif = scr.tile([128, 1], F32, tag="dif")
                        nc.vector.tensor_tensor(
                            dif[:], ibf[:], ridx[m][:], mybir.AluOpType.subtract
                        )
                        sel = scr.tile([128, 1], F32, tag="sel")
                        nc.vector.tensor_tensor(
                            sel[:], dif[:], gt[:], mybir.AluOpType.mult
                        )
                        nc.vector.tensor_tensor(
                            ridx[m][:], sel[:], ridx[m][:], mybir.AluOpType.add
                        )

                        # expand phase for tile m
                        idxu = scr.tile([128, 1], U32, tag="idxu")
                        nc.vector.tensor_copy(idxu[:], ridx[m][:])
                        ru = scr.tile([128, 1], U32, tag="ru")
                        nc.vector.tensor_scalar(
                            ru[:], idxu[:], 6, None, mybir.AluOpType.logical_shift_right
                        )
                        cu = scr.tile([128, 1], U32, tag="cu")
                        nc.vector.tensor_scalar(
                            cu[:], idxu[:], 63, None, mybir.AluOpType.bitwise_and
                        )
                        nr = scr.tile([128, 1], F32, tag="nr")
                        nc.vector.tensor_scalar(
                            nr[:], ru[:], -1.0, None, mybir.AluOpType.mult
                        )
                        ncl = scr.tile([128, 1], F32, tag="ncl")
                        nc.vector.tensor_scalar(
                            ncl[:], cu[:], -1.0, None, mybir.AluOpType.mult
                        )

                        er = scr.tile([128, GRID], F32, tag="er")
                        nc.scalar.activation(
                            er[:], aa[:], mybir.ActivationFunctionType.Square,
                            bias=nr[:], scale=1.0,
                        )
                        nc.scalar.activation(
                            er[:], er[:], mybir.ActivationFunctionType.Exp,
                            scale=-inv_sig2,
                        )
                        ec = scr.tile([128, GRID], F32, tag="ec")
                        nc.scalar.activation(
                            ec[:], aa[:], mybir.ActivationFunctionType.Square,
                            bias=ncl[:], scale=1.0,
                        )
                        nc.scalar.activation(
                            ec[:], ec[:], mybir.ActivationFunctionType.Exp,
                            scale=-inv_sig2,
                        )

                        ot = outp.tile([128, MN], F32, tag="ot")
                        o3 = ot[:].rearrange("p (a b) -> p a b", a=GRID)
                        er_b = er[:].unsqueeze(2).broadcast_to([128, GRID, GRID])
                        ec_b = ec[:].unsqueeze(1).broadcast_to([128, GRID, GRID])
                        nc.vector.tensor_tensor(o3, er_b, ec_b, mybir.AluOpType.mult)

                        nc.sync.dma_start(out_d[msl, :], ot[:])

    nc.compile()
    return nc


def get_nc(inv_sig2: float):
    key = float(inv_sig2)
    if key not in _NC_CACHE:
        _NC_CACHE[key] = _build_kernel(key)
    return _NC_CACHE[key]


def prepare(batch, weights, locations, decay_rate, it):
    batch = np.asarray(batch, dtype=np.float32)
    weights = np.asarray(weights, dtype=np.float32)

    lr = math.exp(-float(it) / float(decay_rate))
    sigma_op = np.float32(SIGMA) * np.float32(lr)
    inv_sig2 = 1.0 / (float(sigma_op) * float(sigma_op))

    wT = weights.T  # [DIM, MN]
    wh = fp32r_round(wT)
    wh16 = bf16(wh)
    wl16 = bf16(wT - wh)
    w2f = (-0.5 * (weights.astype(np.float64) ** 2).sum(axis=1)).astype(np.float32)
    w2a = bf16(w2f)
    w2b = bf16(w2f - w2a.astype(np.float32))
    w2c = bf16(w2f - w2a.astype(np.float32) - w2b.astype(np.float32))
    w2 = np.stack([w2a, w2b, w2c], axis=0)  # [3, MN] bf16
    aa = np.broadcast_to(np.arange(GRID, dtype=np.float32), (128, GRID)).copy()
    ones3 = np.ones((3, 128), dtype=ml_dtypes.bfloat16)

    in_maps = []
    for c in range(N_CORES):
        sT = batch[c * B_SHARD : (c + 1) * B_SHARD, :].T  # [DIM, B_SHARD]
        sh = fp32r_round(sT)
        in_maps.append(
            {
                "bh": sh,
                "bh16": bf16(sh),
                "bl16": bf16(sT - sh),
                "wh": wh,
                "wh16": wh16,
                "wl16": wl16,
                "w2": w2,
                "aa": aa,
                "ones": ones3,
            }
        )
    return inv_sig2, in_maps


def run(inputs, **spmd_kwargs):
    inv_sig2, in_maps = prepare(**inputs)
    nc = get_nc(inv_sig2)
    res = run_bass_kernel_spmd(
        nc, in_maps, core_ids=list(range(N_CORES)), **spmd_kwargs
    )
    out = np.concatenate([r["out"] for r in res.results], axis=0)
    return out, res


def kernel(batch, weights, locations, decay_rate, it):
    out, _ = run(
        dict(
            batch=batch,
            weights=weights,
            locations=locations,
            decay_rate=decay_rate,
            it=it,
        )
    )
    return out
